# revision 1
# baseline (speedup 1.0000x reference)
"""Deformable attention Bass/Trainium2 kernel (8-core SPMD, data-parallel over batch).

Self-contained: hardcodes shapes B=16, Q=900, D=256, NH=8, NP=4, H=W=100.

Per batch on device:
  - project query -> sampling locations / softmax attention, in a
    (head, point, corner)-replicated partition layout so all weight math is
    plain elementwise DVE ops
  - transpose input tiles on PE, project values with W_val (f32r matmuls)
  - write per-head interleaved table C[h][m] = [V_h[m] | V_h[m+100]] to DRAM
  - one SWDGE dma_gather per (batch, head): each 512B descriptor fetches a
    full 2x2 bilinear patch (overlapping elem_step windows)
  - DVE: multiply patches by fused (attn * wx * wy) weights, strided-reduce
  - PE: transpose + output projection
"""
import numpy as np
from contextlib import ExitStack

import concourse.bass as bass
import concourse.bacc as bacc
import concourse.tile as tile
import concourse.mybir as mybir
from concourse.bass_utils import run_bass_kernel_spmd

F32 = mybir.dt.float32
F32R = mybir.dt.float32r
I16 = mybir.dt.int16

B, Q, D, NH, NP = 16, 900, 256, 8, 4
GRID = 100
HW = GRID * GRID            # 10000
NB = 2                      # batches per core
NCORES = 8
NT = 79                     # 128-row tiles of input (78 full + 1x16)
NR = NT * 128               # 10112 padded C rows
QG = 8                      # q groups of 128 (1024 padded q)
NIDX = QG * NP * 128        # 4096 gather indices per (batch, head)
LAST_ROWS = HW - 78 * 128   # 16
HALF_T = 20                 # input tiles per phase (4 phases)
AL = mybir.AluOpType
EMIT_MODE = "full"
REPEAT = 1


def _ap(base, delta, dims):
    """AP from a (possibly partition-sliced) base AP: keep partition dim,
    replace free dims with [stride, num] pairs (element units)."""
    return bass.AP(base.tensor, base.offset + delta, [list(base.ap[0])] + dims)


def _dram(handle, offset, dims):
    b = handle.ap() if hasattr(handle, "ap") else handle
    return bass.AP(b.tensor, b.offset + offset, dims)


def emit(ctx, tc, outs, ins):
    nc = tc.nc
    (x_d, qt_d, reft_d, wval_d, wout_d, bfin_d, wxb_d, wyb_d, wab_d,
     wc_d, bias_d, refp_d, p1_d, ident_d, sk_d, ones_d) = ins
    out_d = outs[0]

    wp = ctx.enter_context(tc.tile_pool(name="wp", bufs=1))
    xap = ctx.enter_context(tc.tile_pool(name="xap", bufs=2))
    xtp = ctx.enter_context(tc.tile_pool(name="xtp", bufs=1))
    vp = ctx.enter_context(tc.tile_pool(name="vp", bufs=2))
    gp = ctx.enter_context(tc.tile_pool(name="gp", bufs=2))
    qp = ctx.enter_context(tc.tile_pool(name="qp", bufs=6))
    mp = ctx.enter_context(tc.tile_pool(name="mp", bufs=4))
    s1 = ctx.enter_context(tc.tile_pool(name="s1", bufs=2))
    s2 = ctx.enter_context(tc.tile_pool(name="s2", bufs=1))
    ip = ctx.enter_context(tc.tile_pool(name="ip", bufs=2))
    pp = ctx.enter_context(tc.tile_pool(name="pp", bufs=4, space="PSUM"))

    # ---- constants / weights to SBUF ----
    def load_kchunk(handle, cols, dt=F32R):
        # DRAM [2, 128, cols] -> SBUF [128, 2, cols]
        t = wp.tile([128, 2, cols], dt, tag=f"w{handle.name}")
        nc.sync.dma_start(t[:], _dram(handle, 0,
                                      [[cols, 128], [128 * cols, 2],
                                       [1, cols]]).bitcast(dt))
        return t

    wval = load_kchunk(wval_d.tensor, 256)
    wout = load_kchunk(wout_d.tensor, 256)
    wxb = load_kchunk(wxb_d.tensor, 128, F32)
    wyb = load_kchunk(wyb_d.tensor, 128, F32)
    wab = load_kchunk(wab_d.tensor, 128, F32)
    wc = load_kchunk(wc_d.tensor, 64, F32)
    bfin = wp.tile([1, 256], F32)
    nc.sync.dma_start(bfin[:], bfin_d[:])
    biasw = wp.tile([1, 512], F32)
    nc.sync.dma_start(biasw[:], bias_d[:])
    refp = wp.tile([2, 3, 128], F32)
    nc.sync.dma_start(refp[:], refp_d[:])
    p1 = wp.tile([128, 128], F32R)
    nc.sync.dma_start(p1[:], p1_d[:].bitcast(F32R))
    ident = wp.tile([128, 128], F32)
    nc.sync.dma_start(ident[:], ident_d[:])
    sk = wp.tile([128, 4], F32)
    nc.sync.dma_start(sk[:], sk_d[:])
    onesr = wp.tile([1, 1024], F32)
    nc.sync.dma_start(onesr[:], ones_d[:])
    zsb = wp.tile([128, 8, 32], F32)
    nc.vector.memset(zsb[:], 0.0)

    c_dram = [nc.dram_tensor(f"ctab{j}", [NH, NR, 64], F32) for j in range(NB)]
    v_dram = [nc.dram_tensor(f"vtab{j}", [NR + 128, 256], F32) for j in range(NB)]

    NSPL = ((0, 512), (512, Q))

    interps = {}
    for j in [jj for _ in range(REPEAT) for jj in range(NB)]:
        cd = c_dram[j]
        vd = v_dram[j]

        # ---- zero-pad V tail rows (10000 .. NR+128) ----
        for r0 in range(HW, NR + 128, 128):
            r1 = min(r0 + 128, NR + 128)
            nc.scalar.dma_start(
                _dram(vd, r0 * 256, [[256, r1 - r0], [1, 256]]),
                _ap(zsb[0:r1 - r0, :, :], 0, [[1, 256]]))

        # ---- query chain ----
        qt = s2.tile([128, 2, 1024], F32, tag="qt")
        nc.sync.dma_start(qt[:, :, 0:Q],
                          _dram(qt_d.tensor, j * 2 * 128 * Q,
                                [[Q, 128], [128 * Q, 2], [1, Q]]))
        reft = s2.tile([2, 1024], F32, tag="reft")
        nc.sync.dma_start(reft[:, 0:Q],
                          _dram(reft_d.tensor, j * 2 * Q, [[Q, 2], [1, Q]]))

        def proj_big(w_t, bias_off, refrow):
            ps = pp.tile([128, 1024], F32, tag="ps")
            for (n0, n1) in NSPL:
                nc.tensor.matmul(ps[:, n0:n1], w_t[:, 0, :],
                                 qt[:, 0, n0:n1], start=True, stop=False)
                nc.tensor.matmul(ps[:, n0:n1], w_t[:, 1, :],
                                 qt[:, 1, n0:n1], start=False, stop=False)
                nc.tensor.matmul(ps[:, n0:n1],
                                 biasw[:, bias_off:bias_off + 128],
                                 onesr[:, n0:n1], start=False,
                                 stop=refrow is None)
                if refrow is not None:
                    nc.tensor.matmul(ps[:, n0:n1], refp[:, refrow, :],
                                     reft[:, n0:n1], start=False, stop=True)
            return ps

        sxp = proj_big(wxb, 0, 0)
        syp = proj_big(wyb, 128, 1)
        atp = proj_big(wab, 256, None)

        scp = pp.tile([64, 1024], F32, tag="ps")
        for (n0, n1) in NSPL:
            nc.tensor.matmul(scp[:, n0:n1], wc[:, 0, :],
                             qt[:, 0, n0:n1], start=True, stop=False)
            nc.tensor.matmul(scp[:, n0:n1], wc[:, 1, :],
                             qt[:, 1, n0:n1], start=False, stop=False)
            nc.tensor.matmul(scp[:, n0:n1], biasw[:, 384:448],
                             onesr[:, n0:n1], start=False, stop=False)
            nc.tensor.matmul(scp[:, n0:n1], refp[:, 2, 0:64],
                             reft[:, n0:n1], start=False, stop=True)

        # bilinear weights in big (h,pt,c)-replicated layout [128, Q]
        MAGIC = 8388608.0  # 2**23: s+MAGIC-MAGIC = round-to-nearest-int(s)

        def frac_of(psum_src, np_, tag):
            # s = 99*clip01(src); returns frac tile = s - floor(s)
            s_ = qp.tile([128, 928], F32, tag=tag)
            nc.vector.tensor_scalar(out=s_[:np_, 0:Q], in0=psum_src[0:np_, 0:Q],
                                    scalar1=0.0, scalar2=1.0, op0=AL.max, op1=AL.min)
            nc.vector.tensor_scalar(out=s_[:np_, 0:Q], in0=s_[:np_, 0:Q],
                                    scalar1=99.0, scalar2=None, op0=AL.mult)
            r_ = qp.tile([128, 928], F32, tag=tag)
            nc.vector.tensor_scalar(out=r_[:np_, 0:Q], in0=s_[:np_, 0:Q],
                                    scalar1=MAGIC, scalar2=MAGIC,
                                    op0=AL.add, op1=AL.subtract)
            g_ = qp.tile([128, 928], F32, tag=tag)
            nc.vector.tensor_tensor(out=g_[:np_, 0:Q], in0=r_[:np_, 0:Q],
                                    in1=s_[:np_, 0:Q], op=AL.is_gt)
            # floor = r - g  (reuse r_)
            nc.vector.tensor_tensor(out=r_[:np_, 0:Q], in0=r_[:np_, 0:Q],
                                    in1=g_[:np_, 0:Q], op=AL.subtract)
            # frac = s - floor (reuse s_)
            nc.vector.tensor_tensor(out=s_[:np_, 0:Q], in0=s_[:np_, 0:Q],
                                    in1=r_[:np_, 0:Q], op=AL.subtract)
            return s_, r_

        fxb, _ = frac_of(sxp, 128, "qc")
        wxw = qp.tile([128, 928], F32, tag="qc")
        nc.vector.tensor_scalar(out=wxw[:, 0:Q], in0=fxb[:, 0:Q], scalar1=sk[:, 0:1],
                                scalar2=sk[:, 1:2], op0=AL.mult, op1=AL.add)
        fyb, _ = frac_of(syp, 128, "qc")
        wyw = qp.tile([128, 928], F32, tag="qc")
        nc.vector.tensor_scalar(out=wyw[:, 0:Q], in0=fyb[:, 0:Q], scalar1=sk[:, 2:3],
                                scalar2=sk[:, 3:4], op0=AL.mult, op1=AL.add)

        eb = qp.tile([128, 928], F32R, tag="qc")
        nc.scalar.activation(eb[:, 0:Q], atp[:, 0:Q], mybir.ActivationFunctionType.Exp)
        dnp_ = pp.tile([128, 1024], F32, tag="ps")
        for (n0, n1) in NSPL:
            nc.tensor.matmul(dnp_[:, n0:n1], p1[:],
                             eb[:, n0:n1], start=True, stop=True)
        rb = qp.tile([128, 928], F32, tag="qc")
        nc.vector.reciprocal(rb[:, 0:Q], dnp_[:, 0:Q])

        t1 = qp.tile([128, 928], F32, tag="qc")
        nc.vector.tensor_tensor(out=t1[:, 0:Q], in0=eb[:, 0:Q], in1=rb[:, 0:Q],
                                op=AL.mult)
        t2 = qp.tile([128, 928], F32, tag="qc")
        nc.vector.tensor_tensor(out=t2[:, 0:Q], in0=t1[:, 0:Q], in1=wxw[:, 0:Q],
                                op=AL.mult)
        u2 = s2.tile([128, 1024], F32, tag="u2")
        nc.vector.tensor_tensor(out=u2[:, 0:Q], in0=t2[:, 0:Q], in1=wyw[:, 0:Q],
                                op=AL.mult)
        nc.vector.memset(u2[:, Q:1024], 0.0)

        # compact m = floor(99*clip(sy))*100 + floor(99*clip(sx))  [32, Q]
        def floor99(psrc, base):
            s_ = mp.tile([32, 928], F32, tag="mm")
            nc.vector.tensor_scalar(out=s_[:, 0:Q], in0=psrc[base:base + 32, 0:Q],
                                    scalar1=0.0, scalar2=1.0, op0=AL.max, op1=AL.min)
            nc.vector.tensor_scalar(out=s_[:, 0:Q], in0=s_[:, 0:Q],
                                    scalar1=99.0, scalar2=None, op0=AL.mult)
            r_ = mp.tile([32, 928], F32, tag="mm")
            nc.vector.tensor_scalar(out=r_[:, 0:Q], in0=s_[:, 0:Q],
                                    scalar1=MAGIC, scalar2=MAGIC,
                                    op0=AL.add, op1=AL.subtract)
            g_ = mp.tile([32, 928], F32, tag="mm")
            nc.vector.tensor_tensor(out=g_[:, 0:Q], in0=r_[:, 0:Q],
                                    in1=s_[:, 0:Q], op=AL.is_gt)
            nc.vector.tensor_tensor(out=r_[:, 0:Q], in0=r_[:, 0:Q],
                                    in1=g_[:, 0:Q], op=AL.subtract)
            return r_

        cflx = floor99(scp, 0)
        cfly = floor99(scp, 32)
        mf = mp.tile([32, 1024], F32, tag="mf")
        nc.vector.scalar_tensor_tensor(out=mf[:, 0:Q], in0=cfly[:, 0:Q],
                                       scalar=100.0, in1=cflx[:, 0:Q],
                                       op0=AL.mult, op1=AL.add)
        nc.vector.memset(mf[:, Q:1024], 0.0)

        # m -> transpose -> int16 -> wrapped+replicated gather indices
        mtps = pp.tile([128, 8, 32], F32, tag="ps")
        for ch in range(QG):
            nc.tensor.transpose(mtps[:, ch, :], mf[:, ch * 128:(ch + 1) * 128],
                                ident[0:32, 0:32])
        mti = s1.tile([128, 8, 32], I16, tag="mti")
        nc.vector.tensor_copy(mti[:], mtps[:])
        mti2 = s1.tile([128, 8, 32], I16, tag="mti2")
        nc.sync.dma_start(mti2[0:112, :, :], mti[16:128, :, :])
        idxall = s1.tile([128, 8, 256], I16, tag="idx")
        for phi in range(8):
            src_t = mti if phi % 2 == 0 else mti2
            bp = 32 * (phi // 2)
            src = src_t[bp:bp + 16, :, :]
            nc.vector.tensor_copy(
                _ap(idxall[0:16, :, :], phi, [[256, 8], [32, 8], [8, 4]]),
                _ap(src, 0, [[4, 8], [32, 8], [1, 4]]))
        for g in range(1, 8):
            nc.sync.dma_start(idxall[16 * g:16 * g + 16, :, :], idxall[0:16, :, :])

        # u2 -> uT [128 qr, 8 qg, 128 comp]
        utps = pp.tile([128, 8, 128], F32, tag="ps")
        for ch in range(QG):
            nc.tensor.transpose(utps[:, ch, :], u2[:, ch * 128:(ch + 1) * 128],
                                ident[:])
        ut = s2.tile([128, 8, 128], F32, tag="ut")
        nc.vector.tensor_copy(ut[:], utps[:])

        # ---- V pipeline ----
        xb_off = j * HW * 256
        for half in range(4 if EMIT_MODE != "novproj" else 0):
            t0h = half * HALF_T
            t1h = min(t0h + HALF_T, NT)
            xth = xtp.tile([128, 2, HALF_T * 128], F32R, tag="xth")
            g5 = [(s, min(s + 5, t1h)) for s in range(t0h, t1h, 5)]
            g4 = [(s, min(s + 4, t1h)) for s in range(t0h, t1h, 4)]
            xa_map = {}
            for (s, e) in g5:
                xa = xap.tile([128, 5, 256], F32, tag="xa")
                full = e - s if e * 128 <= HW else e - s - 1
                if full > 0:
                    nc.sync.dma_start(
                        xa[:, 0:full, :],
                        _dram(x_d.tensor, xb_off + s * 128 * 256,
                              [[256, 128], [128 * 256, full], [1, 256]]))
                if e * 128 > HW:
                    nc.sync.dma_start(
                        xa[0:LAST_ROWS, full, :],
                        _dram(x_d.tensor, xb_off + (s + full) * 128 * 256,
                              [[256, LAST_ROWS], [1, 256]]))
                for t in range(s, e):
                    xa_map[t] = (xa, t - s)
            for (s, e) in g4:
                xtg = pp.tile([128, 8, 128], F32, tag="ps")
                nfull = e - s if e * 128 <= HW else e - s - 1
                for t in range(s, e):
                    xa, sl = xa_map[t]
                    rows = 128 if (t + 1) * 128 <= HW else LAST_ROWS
                    for k in range(2):
                        nc.tensor.transpose(
                            xtg[:, (t - s) * 2 + k, 0:rows],
                            xa[0:rows, sl, k * 128:(k + 1) * 128],
                            ident[0:rows, 0:rows])
                base = (s - t0h) * 128
                for k in range(2):
                    if nfull > 0:
                        nc.scalar.copy(
                            xth[:, k, base:base + nfull * 128],
                            _ap(xtg[:, 0:nfull, :], k * 128, [[256, nfull], [1, 128]]))
                    if nfull < e - s:
                        nc.scalar.copy(
                            xth[:, k, base + nfull * 128:base + nfull * 128 + LAST_ROWS],
                            _ap(xtg[:, :, :], nfull * 256 + k * 128,
                                [[1, LAST_ROWS]]))
            for (s, e) in g5:
                vsb = vp.tile([128, 5, 256], F32, tag="vsb")
                nfull = e - s if e * 128 <= HW else e - s - 1
                for t in range(s, e):
                    rows = 128 if (t + 1) * 128 <= HW else LAST_ROWS
                    lc = (t - t0h) * 128
                    vps = pp.tile([128, 256], F32, tag="ps")
                    nc.tensor.matmul(vps[0:rows, :],
                                     xth[:, 0, lc:lc + rows],
                                     wval[:, 0, :], start=True, stop=False)
                    nc.tensor.matmul(vps[0:rows, :],
                                     xth[:, 1, lc:lc + rows],
                                     wval[:, 1, :], start=False, stop=True)
                    if t % 2 == 0:
                        nc.vector.tensor_copy(vsb[0:rows, t - s, :], vps[0:rows, :])
                    else:
                        nc.scalar.copy(vsb[0:rows, t - s, :], vps[0:rows, :])
                # write V natural (contiguous 1KB rows)
                if nfull > 0:
                    nc.sync.dma_start(
                        _dram(vd, s * 128 * 256,
                              [[256, 128], [128 * 256, nfull], [1, 256]]),
                        _ap(vsb[:, 0:nfull, :], 0, [[256, nfull], [1, 256]]))
                if nfull < e - s:
                    nc.sync.dma_start(
                        _dram(vd, (s + nfull) * 128 * 256,
                              [[256, LAST_ROWS], [1, 256]]),
                        _ap(vsb[0:LAST_ROWS, nfull, :], 0, [[1, 256]]))

        # ---- build C from V via DRAM->DRAM (per head) ----
        for h in range(NH if EMIT_MODE not in ("novproj", "noc") else 0):
            nc.scalar.dma_start(
                _dram(cd, h * NR * 64, [[64, NR], [1, 32]]),
                _dram(vd, h * 32, [[256, NR], [1, 32]]))
            nc.scalar.dma_start(
                _dram(cd, h * NR * 64 + 32, [[64, NR], [1, 32]]),
                _dram(vd, 100 * 256 + h * 32, [[256, NR], [1, 32]]))

        # ---- gathers + weighted combine ----
        interp = ip.tile([128, 8, 256], F32, tag="interp")
        if EMIT_MODE != "full":
            nc.vector.memset(interp[:], 0.0)
        for h in range(0 if EMIT_MODE in ("novproj", "nogather") else NH):
            g = gp.tile([128, 32, 128], F32, tag="g")
            win = _dram(cd, h * NR * 64, [[64, NR - 1], [1, 128]])
            for sub in range(4):
                nc.gpsimd.dma_gather(
                    out_ap=g[:, sub * 8:(sub + 1) * 8, :], in_ap=win,
                    idxs_ap=idxall[:, h, sub * 64:(sub + 1) * 64],
                    num_idxs=NIDX // 4, num_idxs_reg=NIDX // 4,
                    elem_size=128, elem_step=64)
            if EMIT_MODE == "gatheronly":
                nc.vector.tensor_copy(interp[:, 0, h * 32:(h + 1) * 32],
                                      g[:, 0, 0:32])
                continue
            gb = g[:]
            gmul = bass.AP(gb.tensor, gb.offset,
                           [list(gb.ap[0]), [512, 8], [128, 4], [32, 4], [1, 32]])
            uap = _ap(ut[:, :, :], h * 16, [[128, 8], [4, 4], [1, 4], [0, 32]])
            nc.vector.tensor_tensor(out=gmul, in0=gmul, in1=uap, op=AL.mult)
            gred = bass.AP(gb.tensor, gb.offset,
                           [list(gb.ap[0]), [512, 8], [1, 32], [128, 4], [32, 4]])
            nc.vector.tensor_reduce(
                out=_ap(interp[:, :, :], h * 32, [[256, 8], [1, 32]]),
                in_=gred, axis=mybir.AxisListType.XY, op=AL.add)

        interps[j] = interp

    # ---- output projection (all batches, emitted last) ----
    for j in range(NB):
        interp = interps[j]
        for qg in range(QG):
            mq = 128 if qg < 7 else Q - 7 * 128
            itps = pp.tile([128, 2, 128], F32, tag="ps")
            for k in range(2):
                nc.tensor.transpose(itps[:, k, 0:mq],
                                    interp[0:mq, qg, k * 128:(k + 1) * 128],
                                    ident[0:mq, 0:mq])
            itsb = s1.tile([128, 2, 128], F32R, tag="itsb")
            nc.vector.tensor_copy(itsb[:, :, 0:mq], itps[:, :, 0:mq])
            ops_ = pp.tile([128, 256], F32, tag="ps")
            nc.tensor.matmul(ops_[0:mq, :], itsb[:, 0, 0:mq],
                             wout[:, 0, :], start=True, stop=False)
            nc.tensor.matmul(ops_[0:mq, :], itsb[:, 1, 0:mq],
                             wout[:, 1, :], start=False, stop=False)
            nc.tensor.matmul(ops_[0:mq, :], onesr[:, 0:mq],
                             bfin[:], start=False, stop=True)
            osb = s1.tile([128, 256], F32, tag="osb")
            nc.scalar.copy(osb[0:mq, :], ops_[0:mq, :])
            nc.sync.dma_start(
                _dram(out_d.tensor, (j * Q + qg * 128) * 256, [[256, mq], [1, 256]]),
                osb[0:mq, :])



def host_prep(inputs):
    q = np.asarray(inputs["query"], np.float32)
    rp = np.asarray(inputs["reference_points"], np.float32)
    x = np.asarray(inputs["input_flatten"], np.float32)
    W_off = np.asarray(inputs["W_off"], np.float32)
    b_off = np.asarray(inputs["b_off"], np.float32)
    W_attn = np.asarray(inputs["W_attn"], np.float32)
    b_attn = np.asarray(inputs["b_attn"], np.float32)
    W_val = np.asarray(inputs["W_val"], np.float32)
    b_val = np.asarray(inputs["b_val"], np.float32)
    W_out = np.asarray(inputs["W_out"], np.float32)
    b_out = np.asarray(inputs["b_out"], np.float32)
    assert int(inputs["h"]) == GRID and int(inputs["w"]) == GRID

    p = np.arange(128)
    hh, pt, c = p // 16, (p % 16) // 4, p % 4
    colx = (hh * NP + pt) * 2
    coly = colx + 1
    cola = hh * NP + pt

    wxb = np.ascontiguousarray(W_off[:, colx].reshape(2, 128, 128))
    wyb = np.ascontiguousarray(W_off[:, coly].reshape(2, 128, 128))
    wab = np.ascontiguousarray(W_attn[:, cola].reshape(2, 128, 128))
    jj = np.arange(32)
    wc = np.concatenate([W_off[:, jj * 2], W_off[:, jj * 2 + 1]], axis=1)
    wc = np.ascontiguousarray(wc.reshape(2, 128, 64))
    bias = np.zeros((1, 512), np.float32)
    bias[0, 0:128] = b_off[colx]
    bias[0, 128:256] = b_off[coly]
    bias[0, 256:384] = b_attn[cola]
    bias[0, 384:416] = b_off[jj * 2]
    bias[0, 416:448] = b_off[jj * 2 + 1]
    refp = np.zeros((2, 3, 128), np.float32)
    refp[0, 0, :] = 1.0
    refp[1, 1, :] = 1.0
    refp[0, 2, 0:32] = 1.0
    refp[1, 2, 32:64] = 1.0
    p1 = (p[:, None] // 16 == p[None, :] // 16).astype(np.float32)
    ident = np.eye(128, dtype=np.float32)
    cx, cy = c // 2, c % 2
    sk = np.stack([
        np.where(cx == 0, -4.0, 4.0),
        np.where(cx == 0, 4.0, 0.0),
        np.where(cy == 0, -1.0, 1.0),
        np.where(cy == 0, 1.0, 0.0),
    ], axis=1).astype(np.float32)

    wval = np.ascontiguousarray(W_val.reshape(2, 128, 256))
    wout = np.ascontiguousarray(W_out.reshape(2, 128, 256))
    bfin = (b_val @ W_out + b_out).reshape(1, 256).astype(np.float32)

    shared = dict(wval=wval, wout=wout, bfin=bfin, wxb=wxb, wyb=wyb, wab=wab,
                  wc=wc, bias=bias, refp=refp, p1=p1, ident=ident, sk=sk,
                  ones=np.ones((1, 1024), np.float32))
    maps = []
    for k in range(NCORES):
        sl = slice(k * NB, (k + 1) * NB)
        m = dict(shared)
        m["x"] = np.ascontiguousarray(x[sl])
        m["qt"] = np.ascontiguousarray(q[sl].transpose(0, 2, 1)).reshape(NB, 2, 128, Q)
        m["reft"] = np.ascontiguousarray(rp[sl].transpose(0, 2, 1))
        maps.append(m)
    return maps


IN_SPECS = [
    ("x", [NB, HW, 256], F32),
    ("qt", [NB, 2, 128, Q], F32),
    ("reft", [NB, 2, Q], F32),
    ("wval", [2, 128, 256], F32),
    ("wout", [2, 128, 256], F32),
    ("bfin", [1, 256], F32),
    ("wxb", [2, 128, 128], F32),
    ("wyb", [2, 128, 128], F32),
    ("wab", [2, 128, 128], F32),
    ("wc", [2, 128, 64], F32),
    ("bias", [1, 512], F32),
    ("refp", [2, 3, 128], F32),
    ("p1", [128, 128], F32),
    ("ident", [128, 128], F32),
    ("sk", [128, 4], F32),
    ("ones", [1, 1024], F32),
]


def build_nc():
    nc = bacc.Bacc("TRN2", target_bir_lowering=False, debug=False,
                   num_devices=NCORES)
    ins = [nc.dram_tensor(name, shape, dt, kind="ExternalInput").ap()
           for name, shape, dt in IN_SPECS]
    out = nc.dram_tensor("out", [NB, Q, 256], F32, kind="ExternalOutput").ap()
    with tile.TileContext(nc) as tc:
        with ExitStack() as ctx:
            emit(ctx, tc, [out], ins)
    nc.compile()
    return nc


_NC_CACHE = {}


def kernel(**inputs):
    if "nc" not in _NC_CACHE:
        _NC_CACHE["nc"] = build_nc()
    nc = _NC_CACHE["nc"]
    maps = host_prep(inputs)
    res = run_bass_kernel_spmd(nc, maps, list(range(NCORES)))
    out = np.concatenate([res.results[k]["out"] for k in range(NCORES)], axis=0)
    return np.ascontiguousarray(out.reshape(B, Q, D)).astype(np.float32)



# revision 55
# speedup vs baseline: 1.7404x; 1.7404x over previous
"""Deformable attention Bass/Trainium2 kernel (8-core SPMD, data-parallel over batch).

Self-contained: hardcodes shapes B=16, Q=900, D=256, NH=8, NP=4, H=W=100.

Per core (2 batches), pipelined across engines:
  - query chains for both batches up front (sampling locs / softmax attn in a
    (head, point, corner)-replicated layout; all weight math on PE+DVE)
  - V projection on PE; per-head interleaved table C[h][m] = [V_h[m] | V_h[m+100]]
    written DIRECTLY from SBUF (no intermediate V table in DRAM)
  - gathers via SWDGE prepare_only + trigger_dma so GpSimd never blocks on DMA
    completion; transfers stream on the hardware queues and overlap with the
    next batch's V pipeline
  - DVE: multiply patches by fused (attn * wx * wy) weights, strided-reduce
  - PE: transpose + output projection at the end
"""
import numpy as np
from contextlib import ExitStack

import concourse.bass as bass
import concourse.bacc as bacc
import concourse.tile as tile
import concourse.mybir as mybir
from concourse.bass_utils import run_bass_kernel_spmd

F32 = mybir.dt.float32
F32R = mybir.dt.float32r
BF16 = mybir.dt.bfloat16
I16 = mybir.dt.int16

B, Q, D, NH, NP = 16, 900, 256, 8, 4
GRID = 100
HW = GRID * GRID            # 10000
NB = 2                      # batches per core
NCORES = 8
NT = 79                     # 128-row tiles of input (78 full + 1x16)
NR = NT * 128               # 10112 padded C rows
QG = 8                      # q groups of 128 (1024 padded q)
NIDX = QG * NP * 128        # 4096 gather indices per (batch, head)
LAST_ROWS = HW - 78 * 128   # 16
HALF_T = 20                 # input tiles per phase (4 phases)
AL = mybir.AluOpType
NQUEUES = 4
DMA_SCRATCH = 16384


def _ap(base, delta, dims):
    """AP from a (possibly partition-sliced) base AP: keep partition dim,
    replace free dims with [stride, num] pairs (element units)."""
    return bass.AP(base.tensor, base.offset + delta, [list(base.ap[0])] + dims)


def _dram(handle, offset, dims):
    b = handle.ap() if hasattr(handle, "ap") else handle
    return bass.AP(b.tensor, b.offset + offset, dims)


def emit(ctx, tc, outs, ins):
    nc = tc.nc
    (x_d, qt_d, reft_d, wval_d, wout_d, bfin_d, wxb_d, wyb_d, wab_d,
     wc_d, bias_d, refp_d, p1_d, ident_d, sk_d, ones_d) = ins
    out_d = outs[0]

    wp = ctx.enter_context(tc.tile_pool(name="wp", bufs=1))
    xap = ctx.enter_context(tc.tile_pool(name="xap", bufs=2))
    xtp = ctx.enter_context(tc.tile_pool(name="xtp", bufs=1))
    vp = ctx.enter_context(tc.tile_pool(name="vp", bufs=2))
    gp = ctx.enter_context(tc.tile_pool(name="gp", bufs=3))
    gmp = ctx.enter_context(tc.tile_pool(name="gmp", bufs=2))
    qp = ctx.enter_context(tc.tile_pool(name="qp", bufs=5))
    mp = ctx.enter_context(tc.tile_pool(name="mp", bufs=4))
    s1 = ctx.enter_context(tc.tile_pool(name="s1", bufs=2))
    s2 = ctx.enter_context(tc.tile_pool(name="s2", bufs=1))
    ip = ctx.enter_context(tc.tile_pool(name="ip", bufs=1))
    pp = ctx.enter_context(tc.tile_pool(name="pp", bufs=4, space="PSUM"))

    # ---- constants / weights to SBUF ----
    def load_kchunk(handle, cols, dt=F32R):
        # DRAM [2, 128, cols] -> SBUF [128, 2, cols]
        t = wp.tile([128, 2, cols], dt, tag=f"w{handle.name}")
        nc.sync.dma_start(t[:], _dram(handle, 0,
                                      [[cols, 128], [128 * cols, 2],
                                       [1, cols]]).bitcast(dt))
        return t

    wval = load_kchunk(wval_d.tensor, 256)
    wout = load_kchunk(wout_d.tensor, 256)
    wxb = load_kchunk(wxb_d.tensor, 128, F32)
    wyb = load_kchunk(wyb_d.tensor, 128, F32)
    wab = load_kchunk(wab_d.tensor, 128, F32)
    wc = load_kchunk(wc_d.tensor, 64, F32)
    bfin = wp.tile([1, 256], F32)
    nc.sync.dma_start(bfin[:], bfin_d[:])
    biasw = wp.tile([1, 512], F32)
    nc.sync.dma_start(biasw[:], bias_d[:])
    refp = wp.tile([2, 3, 128], F32)
    nc.sync.dma_start(refp[:], refp_d[:])
    p1 = wp.tile([128, 128], F32R)
    nc.sync.dma_start(p1[:], p1_d[:].bitcast(F32R))
    ident = wp.tile([128, 128], F32)
    nc.sync.dma_start(ident[:], ident_d[:])
    sk = wp.tile([128, 4], F32)
    nc.sync.dma_start(sk[:], sk_d[:])
    onesr = wp.tile([1, 1024], F32)
    nc.sync.dma_start(onesr[:], ones_d[:])
    zsb = wp.tile([128, 64], F32)
    nc.vector.memset(zsb[:], 0.0)

    c_dram = [nc.dram_tensor(f"ctab{j}", [NH, NR, 128], BF16) for j in range(NB)]
    # bf16 copy of W_val for the 2x-rate V matmuls
    wvalb = wp.tile([128, 2, 256], BF16)
    nc.scalar.copy(wvalb[:], wval[:].bitcast(F32))
    # one dedicated completion sem per gather prep: shared sems are unsafe
    # (per-prep +16 updates complete out of order across preps/queues)
    dma_sems = [nc.alloc_semaphore(f"swdge_dma{i}")
                for i in range(NB * NH * NP)]

    NSPL = ((0, 512), (512, Q))
    MAGIC = 8388608.0  # 2**23: s+MAGIC-MAGIC = round-to-nearest-int(s)

    # =========== query chains (both batches up front) ===========
    uts = {}
    idxs = {}
    for j in range(NB):
        qt = s2.tile([128, 2, 1024], F32, tag="qt")
        nc.sync.dma_start(qt[:, :, 0:Q],
                          _dram(qt_d.tensor, j * 2 * 128 * Q,
                                [[Q, 128], [128 * Q, 2], [1, Q]]))
        reft = s2.tile([2, 1024], F32, tag="reft")
        nc.sync.dma_start(reft[:, 0:Q],
                          _dram(reft_d.tensor, j * 2 * Q, [[Q, 2], [1, Q]]))

        def proj_big(w_t, bias_off, refrow):
            ps = pp.tile([128, 1024], F32, tag="ps")
            for (n0, n1) in NSPL:
                nc.tensor.matmul(ps[:, n0:n1], w_t[:, 0, :],
                                 qt[:, 0, n0:n1], start=True, stop=False)
                nc.tensor.matmul(ps[:, n0:n1], w_t[:, 1, :],
                                 qt[:, 1, n0:n1], start=False, stop=False)
                nc.tensor.matmul(ps[:, n0:n1],
                                 biasw[:, bias_off:bias_off + 128],
                                 onesr[:, n0:n1], start=False,
                                 stop=refrow is None)
                if refrow is not None:
                    nc.tensor.matmul(ps[:, n0:n1], refp[:, refrow, :],
                                     reft[:, n0:n1], start=False, stop=True)
            return ps

        sxp = proj_big(wxb, 0, 0)
        syp = proj_big(wyb, 128, 1)
        atp = proj_big(wab, 256, None)

        scp = pp.tile([64, 1024], F32, tag="ps")
        for (n0, n1) in NSPL:
            nc.tensor.matmul(scp[:, n0:n1], wc[:, 0, :],
                             qt[:, 0, n0:n1], start=True, stop=False)
            nc.tensor.matmul(scp[:, n0:n1], wc[:, 1, :],
                             qt[:, 1, n0:n1], start=False, stop=False)
            nc.tensor.matmul(scp[:, n0:n1], biasw[:, 384:448],
                             onesr[:, n0:n1], start=False, stop=False)
            nc.tensor.matmul(scp[:, n0:n1], refp[:, 2, 0:64],
                             reft[:, n0:n1], start=False, stop=True)

        # bilinear weights in big (h,pt,c)-replicated layout [128, Q]
        def frac_of(psum_src, np_, tag):
            # s = 99*clip01(src); returns frac tile = s - floor(s)
            s_ = qp.tile([128, 928], F32, tag=tag)
            nc.vector.tensor_scalar(out=s_[:np_, 0:Q], in0=psum_src[0:np_, 0:Q],
                                    scalar1=0.0, scalar2=1.0, op0=AL.max, op1=AL.min)
            nc.vector.tensor_scalar(out=s_[:np_, 0:Q], in0=s_[:np_, 0:Q],
                                    scalar1=99.0, scalar2=None, op0=AL.mult)
            r_ = qp.tile([128, 928], F32, tag=tag)
            nc.vector.tensor_scalar(out=r_[:np_, 0:Q], in0=s_[:np_, 0:Q],
                                    scalar1=MAGIC, scalar2=MAGIC,
                                    op0=AL.add, op1=AL.subtract)
            g_ = qp.tile([128, 928], F32, tag=tag)
            nc.vector.tensor_tensor(out=g_[:np_, 0:Q], in0=r_[:np_, 0:Q],
                                    in1=s_[:np_, 0:Q], op=AL.is_gt)
            # floor = r - g  (reuse r_)
            nc.vector.tensor_tensor(out=r_[:np_, 0:Q], in0=r_[:np_, 0:Q],
                                    in1=g_[:np_, 0:Q], op=AL.subtract)
            # frac = s - floor (reuse s_)
            nc.vector.tensor_tensor(out=s_[:np_, 0:Q], in0=s_[:np_, 0:Q],
                                    in1=r_[:np_, 0:Q], op=AL.subtract)
            return s_, r_

        fxb, _ = frac_of(sxp, 128, "qc")
        wxw = qp.tile([128, 928], F32, tag="qc")
        nc.vector.tensor_scalar(out=wxw[:, 0:Q], in0=fxb[:, 0:Q], scalar1=sk[:, 0:1],
                                scalar2=sk[:, 1:2], op0=AL.mult, op1=AL.add)
        fyb, _ = frac_of(syp, 128, "qc")
        wyw = qp.tile([128, 928], F32, tag="qc")
        nc.vector.tensor_scalar(out=wyw[:, 0:Q], in0=fyb[:, 0:Q], scalar1=sk[:, 2:3],
                                scalar2=sk[:, 3:4], op0=AL.mult, op1=AL.add)

        eb = qp.tile([128, 928], F32R, tag="qc")
        nc.scalar.activation(eb[:, 0:Q], atp[:, 0:Q], mybir.ActivationFunctionType.Exp)
        dnp_ = pp.tile([128, 1024], F32, tag="ps")
        for (n0, n1) in NSPL:
            nc.tensor.matmul(dnp_[:, n0:n1], p1[:],
                             eb[:, n0:n1], start=True, stop=True)
        rb = qp.tile([128, 928], F32, tag="qc")
        nc.vector.reciprocal(rb[:, 0:Q], dnp_[:, 0:Q])

        t1 = qp.tile([128, 928], F32, tag="qc")
        nc.vector.tensor_tensor(out=t1[:, 0:Q], in0=eb[:, 0:Q], in1=rb[:, 0:Q],
                                op=AL.mult)
        t2 = qp.tile([128, 928], F32, tag="qc")
        nc.vector.tensor_tensor(out=t2[:, 0:Q], in0=t1[:, 0:Q], in1=wxw[:, 0:Q],
                                op=AL.mult)
        u2 = s2.tile([128, 1024], F32, tag="u2", bufs=1)
        nc.vector.tensor_tensor(out=u2[:, 0:Q], in0=t2[:, 0:Q], in1=wyw[:, 0:Q],
                                op=AL.mult)
        nc.vector.memset(u2[:, Q:1024], 0.0)

        # compact m = floor(99*clip(sy))*100 + floor(99*clip(sx))  [32, Q]
        def floor99(psrc, base):
            s_ = mp.tile([32, 928], F32, tag="mm")
            nc.vector.tensor_scalar(out=s_[:, 0:Q], in0=psrc[base:base + 32, 0:Q],
                                    scalar1=0.0, scalar2=1.0, op0=AL.max, op1=AL.min)
            nc.vector.tensor_scalar(out=s_[:, 0:Q], in0=s_[:, 0:Q],
                                    scalar1=99.0, scalar2=None, op0=AL.mult)
            r_ = mp.tile([32, 928], F32, tag="mm")
            nc.vector.tensor_scalar(out=r_[:, 0:Q], in0=s_[:, 0:Q],
                                    scalar1=MAGIC, scalar2=MAGIC,
                                    op0=AL.add, op1=AL.subtract)
            g_ = mp.tile([32, 928], F32, tag="mm")
            nc.vector.tensor_tensor(out=g_[:, 0:Q], in0=r_[:, 0:Q],
                                    in1=s_[:, 0:Q], op=AL.is_gt)
            nc.vector.tensor_tensor(out=r_[:, 0:Q], in0=r_[:, 0:Q],
                                    in1=g_[:, 0:Q], op=AL.subtract)
            return r_

        cflx = floor99(scp, 0)
        cfly = floor99(scp, 32)
        mf = mp.tile([32, 1024], F32, tag="mf", bufs=2)
        nc.vector.scalar_tensor_tensor(out=mf[:, 0:Q], in0=cfly[:, 0:Q],
                                       scalar=100.0, in1=cflx[:, 0:Q],
                                       op0=AL.mult, op1=AL.add)
        nc.vector.memset(mf[:, Q:1024], 0.0)

        # m -> transpose -> int16 -> wrapped+replicated gather indices
        mtps = pp.tile([128, 8, 32], F32, tag="ps")
        for ch in range(QG):
            nc.tensor.transpose(mtps[:, ch, :], mf[:, ch * 128:(ch + 1) * 128],
                                ident[0:32, 0:32])
        mti = s1.tile([128, 8, 32], I16, tag="mti", bufs=1)
        nc.vector.tensor_copy(mti[:], mtps[:])
        mti2 = s1.tile([128, 8, 32], I16, tag="mti2", bufs=1)
        nc.sync.dma_start(mti2[0:112, :, :], mti[16:128, :, :])
        idxall = s1.tile([128, 8, 256], I16, tag=f"idx_{j}", bufs=1)
        for phi in range(8):
            src_t = mti if phi % 2 == 0 else mti2
            bp = 32 * (phi // 2)
            src = src_t[bp:bp + 16, :, :]
            nc.vector.tensor_copy(
                _ap(idxall[0:16, :, :], phi, [[256, 8], [32, 8], [8, 4]]),
                _ap(src, 0, [[4, 8], [32, 8], [1, 4]]))
        for g in range(1, 8):
            nc.sync.dma_start(idxall[16 * g:16 * g + 16, :, :], idxall[0:16, :, :])

        # u2 -> uT [128 qr, 8 qg, 128 comp]
        utps = pp.tile([128, 8, 128], F32, tag="ps")
        for ch in range(QG):
            nc.tensor.transpose(utps[:, ch, :], u2[:, ch * 128:(ch + 1) * 128],
                                ident[:])
        ut = s2.tile([128, 8, 128], BF16, tag=f"ut_{j}", bufs=1)
        nc.vector.tensor_copy(ut[:], utps[:])
        uts[j] = ut
        idxs[j] = idxall

    # =========== zero C pad rows (gather targets beyond the grid) ===========
    for j in range(NB):
        cd = c_dram[j]
        for h in range(NH):
            for (r0, n) in ((10000, NR - 10000),):
                nc.sync.dma_start(
                    _dram(cd, (h * NR + r0) * 128, [[128, n], [1, 128]]),
                    zsb[0:n, :].bitcast(BF16))

    # =========== V+C pipeline / gathers / combines, software-pipelined ===========
    def v_pipeline(j):
        """V projection + C-row assembly + direct C writes for batch j.

        C row m = [V[m] | V[m+100]] is assembled fully in SBUF: the +100
        shifted rows come from a second PE matmul reading xth at a +100
        column offset (phases overlap by one tile to cover the window).
        C rows are then 256B contiguous per partition, so each (group,
        head) is a single DMA of 256B lines."""
        cd = c_dram[j]
        xb_off = j * HW * 256
        for half in range(4):
            t0h = half * HALF_T
            t1h = min(t0h + HALF_T, NT)
            # one extra overlap tile (except last phase) for the +100 shift
            t1x = min(t1h + 1, NT)
            xth = xtp.tile([128, 2, (HALF_T + 1) * 128], BF16, tag="xth")
            g5 = [(s, min(s + 5, t1x)) for s in range(t0h, t1x, 5)]
            g4 = [(s, min(s + 4, t1x)) for s in range(t0h, t1x, 4)]
            xa_map = {}
            for gi, (s, e) in enumerate(g5):
                # alternate HWDGE queues; keep x off the Pool/SWDGE path
                # (Pool DMAs tick the DMASW lanes that gather-prep
                # IncSwdgeSem reconciliation waits on)
                xeng = nc.sync
                xa = xap.tile([128, 5, 256], F32, tag="xa")
                full = e - s if e * 128 <= HW else e - s - 1
                if full > 0:
                    xeng.dma_start(
                        xa[:, 0:full, :],
                        _dram(x_d.tensor, xb_off + s * 128 * 256,
                              [[256, 128], [128 * 256, full], [1, 256]]))
                if e * 128 > HW:
                    xeng.dma_start(
                        xa[0:LAST_ROWS, full, :],
                        _dram(x_d.tensor, xb_off + (s + full) * 128 * 256,
                              [[256, LAST_ROWS], [1, 256]]))
                for t in range(s, e):
                    xa_map[t] = (xa, t - s)
            for (s, e) in g4:
                xtg = pp.tile([128, 8, 128], F32, tag="ps")
                nfull = e - s if e * 128 <= HW else e - s - 1
                for t in range(s, e):
                    xa, sl = xa_map[t]
                    rows = 128 if (t + 1) * 128 <= HW else LAST_ROWS
                    for k in range(2):
                        nc.tensor.transpose(
                            xtg[:, (t - s) * 2 + k, 0:rows],
                            xa[0:rows, sl, k * 128:(k + 1) * 128],
                            ident[0:rows, 0:rows])
                base = (s - t0h) * 128
                for k in range(2):
                    if nfull > 0:
                        nc.scalar.copy(
                            xth[:, k, base:base + nfull * 128],
                            _ap(xtg[:, 0:nfull, :], k * 128, [[256, nfull], [1, 128]]))
                    if nfull < e - s:
                        nc.scalar.copy(
                            xth[:, k, base + nfull * 128:base + nfull * 128 + LAST_ROWS],
                            _ap(xtg[:, :, :], nfull * 256 + k * 128,
                                [[1, LAST_ROWS]]))
            for (s, e) in [(s, e) for (s, e) in g5 if s < t1h]:
                e = min(e, t1h)
                csb = vp.tile([128, 5, 8, 128], BF16, tag="csb")
                nfull = e - s if e * 128 <= HW else e - s - 1
                for t in range(s, e):
                    m0 = t * 128
                    rows = 128 if (t + 1) * 128 <= HW else LAST_ROWS
                    lc = (t - t0h) * 128
                    sl = t - s
                    # 4 corner row-shifts per tile: +0, +100, +1, +101
                    # (patch order must match the bilinear weight layout)
                    crows = [min(rows, max(0, HW - m0 - off))
                             for off in (0, 100, 1, 101)]
                    vv = pp.tile([128, 1024], F32, tag="ps")
                    for ci, off in enumerate((0, 100, 1, 101)):
                        rk = crows[ci]
                        if rk == 0:
                            continue
                        nc.tensor.matmul(vv[0:rk, ci * 256:(ci + 1) * 256],
                                         xth[:, 0, lc + off:lc + off + rk],
                                         wvalb[:, 0, :], start=True, stop=False)
                        nc.tensor.matmul(vv[0:rk, ci * 256:(ci + 1) * 256],
                                         xth[:, 1, lc + off:lc + off + rk],
                                         wvalb[:, 1, :], start=False, stop=True)
                    if min(crows) == rows:
                        # single merged interleave copy (all 4 corners x 8 heads)
                        # batch 0 halves 1-3: vector is idle there (qchains
                        # done, combines not yet) — offload from scalar
                        if j == 0 and half >= 1:
                            nc.vector.tensor_copy(
                                _ap(csb[0:rows, sl, :, :], 0,
                                    [[128, 8], [32, 4], [1, 32]]),
                                _ap(vv[0:rows, :], 0, [[32, 8], [256, 4], [1, 32]]))
                        else:
                            nc.scalar.copy(
                                _ap(csb[0:rows, sl, :, :], 0,
                                    [[128, 8], [32, 4], [1, 32]]),
                                _ap(vv[0:rows, :], 0, [[32, 8], [256, 4], [1, 32]]))
                    else:
                        # edge tiles: 32-aligned zero fill first, then the
                        # valid-row copy overwrites the aligned overlap
                        for ci in range(4):
                            rk = crows[ci]
                            if rk < rows:
                                z0 = (rk // 32) * 32
                                for p0 in range(z0, rows, 32):
                                    p1 = min(p0 + 32, rows)
                                    nc.scalar.copy(
                                        _ap(csb[p0:p1, sl, :, :], ci * 32,
                                            [[128, 8], [1, 32]]),
                                        _ap(zsb[p0:p1, :], 0, [[0, 8], [1, 32]]))
                            if rk > 0:
                                nc.scalar.copy(
                                    _ap(csb[0:rk, sl, :, :], ci * 32,
                                        [[128, 8], [1, 32]]),
                                    _ap(vv[0:rk, :], ci * 256, [[32, 8], [1, 32]]))
                # C writes: one dma per head over the group's full tiles
                for h in range(NH):
                    hb = h * NR * 128
                    weng = nc.sync
                    if nfull > 0:
                        weng.dma_start(
                            _dram(cd, hb + s * 128 * 128,
                                  [[128, 128], [128 * 128, nfull], [1, 128]]),
                            _ap(csb[:, 0:nfull, :, :], h * 128,
                                [[1024, nfull], [1, 128]]))
                    if nfull < e - s:
                        weng.dma_start(
                            _dram(cd, hb + (s + nfull) * 128 * 128,
                                  [[128, LAST_ROWS], [1, 128]]),
                            _ap(csb[0:LAST_ROWS, nfull, :, :], h * 128,
                                [[1, 128]]))

    def gather_combine(j):
        """Per (head, sub): SWDGE prep+trigger on gpsimd, weighted combine on
        DVE. Emission is interleaved so g-buffer rotation records WAR deps,
        but the gpsimd stream runs ahead of the DVE stream at execution."""
        cd = c_dram[j]
        idxall = idxs[j]
        ut = uts[j]
        interp = ip.tile([128, 8, 256], F32, tag="interp")
        for h in range(NH):
            win = _dram(cd, h * NR * 128, [[128, NR - 1], [1, 128]])
            g = gp.tile([128, 32, 128], BF16, tag="g")
            # emit the head's 4 preps (one per queue) before any trigger so
            # each trigger's wait on its prep's engine tick (~8us sem
            # round-trip) is already satisfied when it executes
            for sub in range(4):
                nc.gpsimd.dma_gather(
                    out_ap=g[:, sub * 8:(sub + 1) * 8, :], in_ap=win,
                    idxs_ap=idxall[:, h, sub * 64:(sub + 1) * 64],
                    num_idxs=NIDX // 4, num_idxs_reg=NIDX // 4,
                    elem_size=128, elem_step=128,
                    prepare_only=True,
                    sem=dma_sems[(j * NH + h) * 4 + sub], queue_num=sub)
            for sub in range(4):
                nc.gpsimd.trigger_dma(count=None, queue_num=sub)
            for sub in range(4):
                # tile's DMASW-lane wait doesn't cover prepare_only preps
                # (they inc the user sem, not the lane sem) — wait explicitly.
                nc.vector.wait_ge(dma_sems[(j * NH + h) * 4 + sub], 16)
            # weighted product to a separate bf16 tile: keeps the reduce off
            # the g-buffer WAR loop (prep h+2 only waits for this TT) and
            # halves the reduce's read bandwidth
            gm = gmp.tile([128, 32, 128], BF16, tag="gm")
            gb = g[:]
            gmul = bass.AP(gb.tensor, gb.offset,
                           [list(gb.ap[0]), [512, 8], [128, 4], [32, 4], [1, 32]])
            gmo = bass.AP(gm.tensor, gm.offset,
                          [list(gm.ap[0]), [512, 8], [128, 4], [32, 4], [1, 32]])
            uap = _ap(ut[:, :, :], h * 16, [[128, 8], [4, 4], [1, 4], [0, 32]])
            nc.vector.tensor_tensor(out=gmo, in0=gmul, in1=uap, op=AL.mult)
            gred = bass.AP(gm.tensor, gm.offset,
                           [list(gm.ap[0]), [512, 8], [1, 32], [128, 4], [32, 4]])
            nc.vector.tensor_reduce(
                out=_ap(interp[:, :, :], h * 32, [[256, 8], [1, 32]]),
                in_=gred, axis=mybir.AxisListType.XY, op=AL.add)
        return interp

    def out_proj(j, interp):
        for qg in range(QG):
            mq = 128 if qg < 7 else Q - 7 * 128
            itps = pp.tile([128, 2, 128], F32, tag="ps")
            for k in range(2):
                nc.tensor.transpose(itps[:, k, 0:mq],
                                    interp[0:mq, qg, k * 128:(k + 1) * 128],
                                    ident[0:mq, 0:mq])
            itsb = s1.tile([128, 2, 128], F32R, tag="itsb", bufs=1)
            nc.vector.tensor_copy(itsb[:, :, 0:mq], itps[:, :, 0:mq])
            ops_ = pp.tile([128, 256], F32, tag="ps")
            nc.tensor.matmul(ops_[0:mq, :], itsb[:, 0, 0:mq],
                             wout[:, 0, :], start=True, stop=False)
            nc.tensor.matmul(ops_[0:mq, :], itsb[:, 1, 0:mq],
                             wout[:, 1, :], start=False, stop=False)
            nc.tensor.matmul(ops_[0:mq, :], onesr[:, 0:mq],
                             bfin[:], start=False, stop=True)
            osb = s1.tile([128, 256], F32, tag="osb", bufs=1)
            nc.scalar.copy(osb[0:mq, :], ops_[0:mq, :])
            nc.sync.dma_start(
                _dram(out_d.tensor, (j * Q + qg * 128) * 256, [[256, mq], [1, 256]]),
                osb[0:mq, :])

    v_pipeline(0)
    v_pipeline(1)
    out_proj(0, gather_combine(0))
    out_proj(1, gather_combine(1))


def host_prep(inputs):
    q = np.asarray(inputs["query"], np.float32)
    rp = np.asarray(inputs["reference_points"], np.float32)
    x = np.asarray(inputs["input_flatten"], np.float32)
    W_off = np.asarray(inputs["W_off"], np.float32)
    b_off = np.asarray(inputs["b_off"], np.float32)
    W_attn = np.asarray(inputs["W_attn"], np.float32)
    b_attn = np.asarray(inputs["b_attn"], np.float32)
    W_val = np.asarray(inputs["W_val"], np.float32)
    b_val = np.asarray(inputs["b_val"], np.float32)
    W_out = np.asarray(inputs["W_out"], np.float32)
    b_out = np.asarray(inputs["b_out"], np.float32)
    assert int(inputs["h"]) == GRID and int(inputs["w"]) == GRID

    p = np.arange(128)
    hh, pt, c = p // 16, (p % 16) // 4, p % 4
    colx = (hh * NP + pt) * 2
    coly = colx + 1
    cola = hh * NP + pt

    wxb = np.ascontiguousarray(W_off[:, colx].reshape(2, 128, 128))
    wyb = np.ascontiguousarray(W_off[:, coly].reshape(2, 128, 128))
    wab = np.ascontiguousarray(W_attn[:, cola].reshape(2, 128, 128))
    jj = np.arange(32)
    wc = np.concatenate([W_off[:, jj * 2], W_off[:, jj * 2 + 1]], axis=1)
    wc = np.ascontiguousarray(wc.reshape(2, 128, 64))
    bias = np.zeros((1, 512), np.float32)
    bias[0, 0:128] = b_off[colx]
    bias[0, 128:256] = b_off[coly]
    bias[0, 256:384] = b_attn[cola]
    bias[0, 384:416] = b_off[jj * 2]
    bias[0, 416:448] = b_off[jj * 2 + 1]
    refp = np.zeros((2, 3, 128), np.float32)
    refp[0, 0, :] = 1.0
    refp[1, 1, :] = 1.0
    refp[0, 2, 0:32] = 1.0
    refp[1, 2, 32:64] = 1.0
    p1 = (p[:, None] // 16 == p[None, :] // 16).astype(np.float32)
    ident = np.eye(128, dtype=np.float32)
    cx, cy = c // 2, c % 2
    sk = np.stack([
        np.where(cx == 0, -4.0, 4.0),
        np.where(cx == 0, 4.0, 0.0),
        np.where(cy == 0, -1.0, 1.0),
        np.where(cy == 0, 1.0, 0.0),
    ], axis=1).astype(np.float32)

    wval = np.ascontiguousarray(W_val.reshape(2, 128, 256))
    wout = np.ascontiguousarray(W_out.reshape(2, 128, 256))
    bfin = (b_val @ W_out + b_out).reshape(1, 256).astype(np.float32)

    shared = dict(wval=wval, wout=wout, bfin=bfin, wxb=wxb, wyb=wyb, wab=wab,
                  wc=wc, bias=bias, refp=refp, p1=p1, ident=ident, sk=sk,
                  ones=np.ones((1, 1024), np.float32))
    maps = []
    for k in range(NCORES):
        sl = slice(k * NB, (k + 1) * NB)
        m = dict(shared)
        m["x"] = np.ascontiguousarray(x[sl])
        m["qt"] = np.ascontiguousarray(q[sl].transpose(0, 2, 1)).reshape(NB, 2, 128, Q)
        m["reft"] = np.ascontiguousarray(rp[sl].transpose(0, 2, 1))
        maps.append(m)
    return maps


IN_SPECS = [
    ("x", [NB, HW, 256], F32),
    ("qt", [NB, 2, 128, Q], F32),
    ("reft", [NB, 2, Q], F32),
    ("wval", [2, 128, 256], F32),
    ("wout", [2, 128, 256], F32),
    ("bfin", [1, 256], F32),
    ("wxb", [2, 128, 128], F32),
    ("wyb", [2, 128, 128], F32),
    ("wab", [2, 128, 128], F32),
    ("wc", [2, 128, 64], F32),
    ("bias", [1, 512], F32),
    ("refp", [2, 3, 128], F32),
    ("p1", [128, 128], F32),
    ("ident", [128, 128], F32),
    ("sk", [128, 4], F32),
    ("ones", [1, 1024], F32),
]


def build_nc():
    nc = bacc.Bacc("TRN2", target_bir_lowering=False, debug=False,
                   num_devices=NCORES, num_swdge_queues=NQUEUES,
                   dynamic_dma_scratch_size=DMA_SCRATCH)
    ins = [nc.dram_tensor(name, shape, dt, kind="ExternalInput").ap()
           for name, shape, dt in IN_SPECS]
    out = nc.dram_tensor("out", [NB, Q, 256], F32, kind="ExternalOutput").ap()
    with tile.TileContext(nc) as tc:
        with ExitStack() as ctx:
            emit(ctx, tc, [out], ins)
    nc.compile()
    return nc


_NC_CACHE = {}


def kernel(**inputs):
    if "nc" not in _NC_CACHE:
        _NC_CACHE["nc"] = build_nc()
    nc = _NC_CACHE["nc"]
    maps = host_prep(inputs)
    res = run_bass_kernel_spmd(nc, maps, list(range(NCORES)))
    out = np.concatenate([res.results[k]["out"] for k in range(NCORES)], axis=0)
    return np.ascontiguousarray(out.reshape(B, Q, D)).astype(np.float32)
